# revision 5
# baseline (speedup 1.0000x reference)
"""CrossLayerTranscoder kernel for 8 Trainium2 NeuronCores.

Computation (reference):
    feats[l] = relu(x[l] @ W_enc[l].T)                       [L, T, F]
    recon[l] = sum_{s<=l} feats[s] @ W_dec[s,l].T            [L, T, H]
with L=16, T=2048, H=2048, F=512.

Strategy: data-parallel over the token axis T (256 tokens per core),
weights replicated. All matmuls run in bf16 with fp32 PSUM accumulation.
Host side pre-transposes AND partition-folds the operands so that on-chip
the contraction axis is always the SBUF partition axis and every DMA
descriptor is a multi-KiB contiguous run per partition:
  xT[l]  packed [128, HB, Tc]   from x[l].T   [H, Tc]  (encoder moving)
  wencT  packed [128, HB, F]    from W_enc[l].T [H, F] (encoder stationary)
  wdecT  packed [128, FBLK, H]  from W_dec[s,l].T [F, H] (decoder moving,
                                 tri-packed over the 136 (s<=l) pairs)
Encoder produces feats^T [F, Tc] tiles in PSUM; relu'd bf16 copies stay in
SBUF as decoder stationary operands and are also DMA'd out (host upcasts).
Decoder emits recon[t, h] tiles directly in the natural output layout.
"""

import numpy as np
import ml_dtypes

import concourse.bass as bass
import concourse.mybir as mybir
import concourse.tile as tile
from concourse import bacc
from concourse.bass_utils import run_bass_kernel_spmd

L, T, H, F = 16, 2048, 2048, 512
NCORES = 8
TC = T // NCORES          # 256 tokens per core
HB = H // 128             # 16 h-blocks
FBLK = F // 128           # 4 f-blocks
NB = H // 512             # 4 decoder output column blocks
TT = TC // 128            # 2 token tiles
NPAIR = L * (L + 1) // 2  # 136 (s <= l)

BF16 = mybir.dt.bfloat16
F32 = mybir.dt.float32

_CACHE = {}


def _build_program():
    nc = bacc.Bacc("TRN2", target_bir_lowering=False, debug=False)

    xT = nc.dram_tensor("xT", [L, 128, HB, TC], BF16, kind="ExternalInput")
    wencT = nc.dram_tensor("wencT", [L, 128, HB, F], BF16, kind="ExternalInput")
    wdecT = nc.dram_tensor("wdecT", [NPAIR, 128, FBLK, H], BF16, kind="ExternalInput")
    featsT = nc.dram_tensor("featsT", [L, F, TC], BF16, kind="ExternalOutput")
    recon = nc.dram_tensor("recon", [L, TC, H], F32, kind="ExternalOutput")

    with tile.TileContext(nc) as tc:
        with (
            tc.tile_pool(name="xp", bufs=2) as xp,
            tc.tile_pool(name="wep", bufs=2) as wep,
            tc.tile_pool(name="fbf", bufs=L * FBLK) as fbf,
            tc.tile_pool(name="wdp", bufs=5) as wdp,
            tc.tile_pool(name="rst", bufs=8) as rst,
            tc.tile_pool(name="ps", bufs=8, space="PSUM") as ps,
        ):
            # ---------------- encoder ----------------
            feats_sb = [[None] * FBLK for _ in range(L)]
            for layer in range(L):
                xt = xp.tile([128, HB, TC], BF16, tag="xt")
                nc.sync.dma_start(out=xt, in_=xT[layer])
                we = wep.tile([128, HB, F], BF16, tag="we")
                nc.sync.dma_start(out=we, in_=wencT[layer])
                enc_ps = []
                for fb in range(FBLK):
                    enc_ps.append(ps.tile([128, TC], F32, tag="ps", name="encps"))
                for hb in range(HB):
                    for fb in range(FBLK):
                        nc.tensor.matmul(
                            enc_ps[fb],
                            we[:, hb, fb * 128 : (fb + 1) * 128],
                            xt[:, hb, :],
                            start=(hb == 0),
                            stop=(hb == HB - 1),
                        )
                for fb in range(FBLK):
                    fb_bf = fbf.tile([128, TC], BF16, tag="fbf")
                    nc.scalar.activation(
                        fb_bf, enc_ps[fb], mybir.ActivationFunctionType.Relu
                    )
                    feats_sb[layer][fb] = fb_bf
                    nc.sync.dma_start(
                        out=featsT[layer, fb * 128 : (fb + 1) * 128, :],
                        in_=fb_bf,
                    )

            # ---------------- decoder ----------------
            pair = 0
            for target in range(L):
                dec_ps = [
                    [ps.tile([128, 512], F32, tag="ps", name="decps") for _ in range(NB)]
                    for _ in range(TT)
                ]
                for s in range(target + 1):
                    wd = wdp.tile([128, FBLK, H], BF16, tag="wd")
                    nc.sync.dma_start(out=wd, in_=wdecT[pair])
                    for fb in range(FBLK):
                        for tt in range(TT):
                            lhsT = feats_sb[s][fb][:, tt * 128 : (tt + 1) * 128]
                            for n in range(NB):
                                nc.tensor.matmul(
                                    dec_ps[tt][n],
                                    lhsT,
                                    wd[:, fb, n * 512 : (n + 1) * 512],
                                    start=(s == 0 and fb == 0),
                                    stop=(s == target and fb == FBLK - 1),
                                )
                    pair += 1
                for tt in range(TT):
                    for n in range(NB):
                        r_st = rst.tile([128, 512], F32, tag="rst")
                        nc.vector.tensor_copy(r_st, dec_ps[tt][n])
                        nc.sync.dma_start(
                            out=recon[
                                target,
                                tt * 128 : (tt + 1) * 128,
                                n * 512 : (n + 1) * 512,
                            ],
                            in_=r_st,
                        )

    nc.compile()
    return nc


def _get_program():
    if "nc" not in _CACHE:
        _CACHE["nc"] = _build_program()
    return _CACHE["nc"]


def _prep_inputs(x, W_enc, W_dec):
    """Host-side packing shared by all cores (bf16 cast + partition folds)."""
    bf16 = ml_dtypes.bfloat16
    # W_enc: [L, F, H] -> W_enc.T per layer [H, F] -> packed [L, 128, HB, F]
    wencT = np.ascontiguousarray(
        W_enc.transpose(0, 2, 1).reshape(L, HB, 128, F).transpose(0, 2, 1, 3)
    ).astype(bf16)
    # W_dec tri-pack: pair (l, s<=l) -> W_dec[s,l].T [F, H] -> [128, FBLK, H]
    wdecT = np.empty((NPAIR, 128, FBLK, H), dtype=bf16)
    idx = 0
    for target in range(L):
        for s in range(target + 1):
            wdecT[idx] = (
                W_dec[s, target].T.reshape(FBLK, 128, H).transpose(1, 0, 2)
            )
            idx += 1
    # x: [L, T, H] -> x.T per layer [H, T] -> packed [L, 128, HB, T]
    xP = np.ascontiguousarray(
        x.transpose(0, 2, 1).reshape(L, HB, 128, T).transpose(0, 2, 1, 3)
    ).astype(bf16)
    return xP, wencT, wdecT


def kernel(x, W_enc, W_dec):
    x = np.asarray(x, dtype=np.float32)
    W_enc = np.asarray(W_enc, dtype=np.float32)
    W_dec = np.asarray(W_dec, dtype=np.float32)

    xP, wencT, wdecT = _prep_inputs(x, W_enc, W_dec)

    nc = _get_program()
    in_maps = []
    for c in range(NCORES):
        in_maps.append(
            {
                "xT": np.ascontiguousarray(xP[:, :, :, c * TC : (c + 1) * TC]),
                "wencT": wencT,
                "wdecT": wdecT,
            }
        )

    res = run_bass_kernel_spmd(nc, in_maps, core_ids=list(range(NCORES)))

    feats = np.empty((L, T, F), dtype=np.float32)
    recon = np.empty((L, T, H), dtype=np.float32)
    for c in range(NCORES):
        feats[:, c * TC : (c + 1) * TC, :] = (
            res.results[c]["featsT"].astype(np.float32).transpose(0, 2, 1)
        )
        recon[:, c * TC : (c + 1) * TC, :] = res.results[c]["recon"]
    return feats, recon


# revision 6
# speedup vs baseline: 1.0855x; 1.0855x over previous
"""CrossLayerTranscoder kernel for 8 Trainium2 NeuronCores.

Computation (reference):
    feats[l] = relu(x[l] @ W_enc[l].T)                       [L, T, F]
    recon[l] = sum_{s<=l} feats[s] @ W_dec[s,l].T            [L, T, H]
with L=16, T=2048, H=2048, F=512.

Strategy: data-parallel over the token axis T (256 tokens per core),
weights replicated. All matmuls run in bf16 with fp32 PSUM accumulation.
Host side pre-transposes AND partition-folds the operands so that on-chip
the contraction axis is always the SBUF partition axis and every DMA
descriptor is a multi-KiB contiguous run per partition:
  xT[l]  packed [128, HB, Tc]   from x[l].T     [H, Tc]  (encoder moving)
  wencT  packed [128, HB, F]    from W_enc[l].T [H, F]   (encoder stationary)
  wdecT  packed [128, FBLK, H]  from W_dec[s,l].T [F, H] (decoder moving,
                                 tri-packed over the 136 (s<=l) pairs)

Encoder layer l+2 is interleaved after decoder target l so the PE always
has decoder matmuls to run while encoder DMAs stream; the encoder
accumulates one PSUM bank at a time so that the decoder's 8 banks and the
encoder's share the single 8-slot PSUM pool without deadlock.

DMA streams are split across the three descriptor-generation paths to
avoid head-of-line blocking: W_dec bulk stream on the sync HWDGE ring,
latency-critical encoder loads on the scalar HWDGE ring, and outputs on
the gpsimd SWDGE path.
"""

import numpy as np
import ml_dtypes

import concourse.bass as bass
import concourse.mybir as mybir
import concourse.tile as tile
from concourse import bacc
from concourse.bass_utils import run_bass_kernel_spmd

L, T, H, F = 16, 2048, 2048, 512
NCORES = 8
TC = T // NCORES          # 256 tokens per core
HB = H // 128             # 16 h-blocks
FBLK = F // 128           # 4 f-blocks
NB = H // 512             # 4 decoder output column blocks
TT = TC // 128            # 2 token tiles
NPAIR = L * (L + 1) // 2  # 136 (s <= l)

BF16 = mybir.dt.bfloat16
F32 = mybir.dt.float32

_CACHE = {}


def _build_program():
    nc = bacc.Bacc("TRN2", target_bir_lowering=False, debug=False)

    xT = nc.dram_tensor("xT", [L, 128, HB, TC], BF16, kind="ExternalInput")
    wencT = nc.dram_tensor("wencT", [L, 128, HB, F], BF16, kind="ExternalInput")
    wdecT = nc.dram_tensor("wdecT", [NPAIR, 128, FBLK, H], BF16, kind="ExternalInput")
    featsT = nc.dram_tensor("featsT", [L, 128, FBLK, TC], BF16, kind="ExternalOutput")
    recon = nc.dram_tensor("recon", [L, TC, H], F32, kind="ExternalOutput")

    with tile.TileContext(nc) as tc:
        with (
            tc.tile_pool(name="xp", bufs=2) as xp,
            tc.tile_pool(name="wep", bufs=2) as wep,
            tc.tile_pool(name="fbf", bufs=L) as fbf,
            tc.tile_pool(name="wdp", bufs=5) as wdp,
            tc.tile_pool(name="rst", bufs=3) as rst,
            tc.tile_pool(name="ps", bufs=8, space="PSUM") as ps,
        ):
            feats_sb = [None] * L

            def emit_enc(layer):
                xt = xp.tile([128, HB, TC], BF16, tag="xt", name="xt")
                nc.scalar.dma_start(out=xt, in_=xT[layer])
                we = wep.tile([128, HB, F], BF16, tag="we", name="we")
                nc.scalar.dma_start(out=we, in_=wencT[layer])
                fall = fbf.tile([128, FBLK, TC], BF16, tag="fbf", name="fall")
                for fb in range(FBLK):
                    eps = ps.tile([128, TC], F32, tag="ps", name="encps")
                    for hb in range(HB):
                        nc.tensor.matmul(
                            eps,
                            we[:, hb, fb * 128 : (fb + 1) * 128],
                            xt[:, hb, :],
                            start=(hb == 0),
                            stop=(hb == HB - 1),
                        )
                    nc.scalar.activation(
                        fall[:, fb, :], eps, mybir.ActivationFunctionType.Relu
                    )
                nc.gpsimd.dma_start(out=featsT[layer], in_=fall)
                feats_sb[layer] = fall

            def emit_dec(target, pair0):
                dec_ps = [
                    [
                        ps.tile([128, 512], F32, tag="ps", name="decps")
                        for _ in range(NB)
                    ]
                    for _ in range(TT)
                ]
                for s in range(target + 1):
                    wd = wdp.tile([128, FBLK, H], BF16, tag="wd", name="wd")
                    nc.sync.dma_start(out=wd, in_=wdecT[pair0 + s])
                    for fb in range(FBLK):
                        for tt in range(TT):
                            lhsT = feats_sb[s][:, fb, tt * 128 : (tt + 1) * 128]
                            for n in range(NB):
                                nc.tensor.matmul(
                                    dec_ps[tt][n],
                                    lhsT,
                                    wd[:, fb, n * 512 : (n + 1) * 512],
                                    start=(s == 0 and fb == 0),
                                    stop=(s == target and fb == FBLK - 1),
                                )
                for tt in range(TT):
                    r_st = rst.tile([128, NB * 512], F32, tag="rst", name="rst")
                    for n in range(NB):
                        nc.vector.tensor_copy(
                            r_st[:, n * 512 : (n + 1) * 512], dec_ps[tt][n]
                        )
                    nc.gpsimd.dma_start(
                        out=recon[target, tt * 128 : (tt + 1) * 128, :], in_=r_st
                    )

            emit_enc(0)
            emit_enc(1)
            pair0 = 0
            for target in range(L):
                emit_dec(target, pair0)
                pair0 += target + 1
                if target + 2 < L:
                    emit_enc(target + 2)

    nc.compile()
    return nc


def _get_program():
    if "nc" not in _CACHE:
        _CACHE["nc"] = _build_program()
    return _CACHE["nc"]


def _prep_inputs(x, W_enc, W_dec):
    """Host-side packing shared by all cores (bf16 cast + partition folds)."""
    bf16 = ml_dtypes.bfloat16
    # W_enc: [L, F, H] -> W_enc.T per layer [H, F] -> packed [L, 128, HB, F]
    wencT = np.ascontiguousarray(
        W_enc.transpose(0, 2, 1).reshape(L, HB, 128, F).transpose(0, 2, 1, 3)
    ).astype(bf16)
    # W_dec tri-pack: pair (l, s<=l) -> W_dec[s,l].T [F, H] -> [128, FBLK, H]
    wdecT = np.empty((NPAIR, 128, FBLK, H), dtype=bf16)
    idx = 0
    for target in range(L):
        for s in range(target + 1):
            wdecT[idx] = (
                W_dec[s, target].T.reshape(FBLK, 128, H).transpose(1, 0, 2)
            )
            idx += 1
    # x: [L, T, H] -> x.T per layer [H, T] -> packed [L, 128, HB, T]
    xP = np.ascontiguousarray(
        x.transpose(0, 2, 1).reshape(L, HB, 128, T).transpose(0, 2, 1, 3)
    ).astype(bf16)
    return xP, wencT, wdecT


def kernel(x, W_enc, W_dec):
    x = np.asarray(x, dtype=np.float32)
    W_enc = np.asarray(W_enc, dtype=np.float32)
    W_dec = np.asarray(W_dec, dtype=np.float32)

    xP, wencT, wdecT = _prep_inputs(x, W_enc, W_dec)

    nc = _get_program()
    in_maps = []
    for c in range(NCORES):
        in_maps.append(
            {
                "xT": np.ascontiguousarray(xP[:, :, :, c * TC : (c + 1) * TC]),
                "wencT": wencT,
                "wdecT": wdecT,
            }
        )

    res = run_bass_kernel_spmd(nc, in_maps, core_ids=list(range(NCORES)))

    feats = np.empty((L, T, F), dtype=np.float32)
    recon = np.empty((L, T, H), dtype=np.float32)
    for c in range(NCORES):
        # featsT core layout [L, 128, FBLK, TC] -> [L, TC, FBLK*128]
        f = res.results[c]["featsT"].astype(np.float32)
        feats[:, c * TC : (c + 1) * TC, :] = f.transpose(0, 3, 2, 1).reshape(L, TC, F)
        recon[:, c * TC : (c + 1) * TC, :] = res.results[c]["recon"]
    return feats, recon
